# revision 1
# baseline (speedup 1.0000x reference)
"""ChebNet (K=2, 3 layers + global mean pool + linear) on 8 Trainium2 NeuronCores.

Strategy (pull-based graph parallel):
  - Nodes are dealt (degree-balanced) across 8 cores x 98 tiles of 128 nodes.
  - Each core owns the incoming edges of its nodes (edges sorted by dst tile,
    then by src segment / src for gather locality).
  - Per layer: core computes Y = dinv * (h @ W[1]) for its node shard,
    AllGather -> replicated Y_full [100352, 64] in DRAM.
    SpMM: dma_gather of 256B Y rows by edge src (int16 idx, 4 segments of
    25088 rows), segment-sum via one-hot matmuls accumulating in PSUM
    (one-hot generated on DVE: is_equal(iota_row, dst_rel)).
    Combine: h_next = relu(h @ W[0] + b - dinv * psum_s).
  - Pooling: one-hot (node->graph) matmuls into PSUM, AllReduce, scale by
    1/count, final linear on-device. All cores produce identical output.
"""
import sys

for _p in ("/opt/trn_rl_repo",):
    if _p not in sys.path:
        sys.path.insert(0, _p)

import numpy as np
import concourse.bass as bass
import concourse.mybir as mybir
from concourse import bacc, tile
from concourse.bass_utils import run_bass_kernel_spmd

F32 = mybir.dt.float32
BF16 = mybir.dt.bfloat16
I16 = mybir.dt.int16


class Cfg:
    def __init__(self, N, E, F, H, C, G, ncores=8, tiles=None, block=4, nseg=4):
        self.N, self.E, self.F, self.H, self.C, self.G = N, E, F, H, C, G
        self.ncores = ncores
        # nodes per core, multiple of 128
        npc = -(-N // (ncores * 128)) * 128
        self.NPC = npc
        self.NPAD = npc * ncores
        self.TILES = npc // 128
        self.BLOCK = block
        self.NSEG = nseg
        assert self.NPAD % nseg == 0
        self.SEGROWS = self.NPAD // nseg
        assert self.SEGROWS <= 32767, "segment rows must fit int16"


FULL = Cfg(N=100000, E=1600000, F=64, H=64, C=16, G=64)


# ---------------------------------------------------------------- host prep
def host_prep(cfg, x, edge_index, batch):
    N, G = cfg.N, cfg.G
    ncores, TILES, NPC = cfg.ncores, cfg.TILES, cfg.NPC
    src = np.asarray(edge_index[0], dtype=np.int64)
    dst = np.asarray(edge_index[1], dtype=np.int64)
    batch = np.asarray(batch, dtype=np.int64)

    deg = np.bincount(src, minlength=N).astype(np.float64)
    dinv = np.where(deg > 0, 1.0 / np.sqrt(np.maximum(deg, 1.0)), 0.0).astype(np.float32)

    # ---- deal nodes into (core, tile) bins, balancing in-degree ----
    indeg = np.bincount(dst, minlength=N)
    order = np.argsort(-indeg, kind="stable")
    nbins = ncores * TILES
    k = np.arange(N)
    rnd = k // nbins
    pos = k % nbins
    binid = np.where(rnd % 2 == 0, pos, nbins - 1 - pos)
    slot = rnd
    # dealt global id: bin b -> core = b % ncores, tile = b // ncores
    core_of_bin = binid % ncores
    tile_of_bin = binid // ncores
    g_of_sorted = core_of_bin * NPC + tile_of_bin * 128 + slot
    dealt = np.empty(N, dtype=np.int64)
    dealt[order] = g_of_sorted

    src_g = dealt[src]
    dst_g = dealt[dst]

    # per-node (dealt) attributes
    dinv_d = np.zeros(cfg.NPAD, dtype=np.float32)
    dinv_d[dealt] = dinv
    batch_d = np.full(cfg.NPAD, -1.0, dtype=np.float32)
    batch_d[dealt] = batch.astype(np.float32)
    x_d = np.zeros((cfg.NPAD, cfg.F), dtype=np.float32)
    x_d[dealt] = np.asarray(x, dtype=np.float32)

    # ---- edge organization ----
    ecore = dst_g // NPC
    etile = (dst_g % NPC) // 128
    edrel = dst_g % 128
    eseg = src_g // cfg.SEGROWS
    eidx = (src_g % cfg.SEGROWS).astype(np.int16)

    order_e = np.lexsort((src_g, eseg, etile, ecore))
    ecore, etile, edrel, eseg, eidx = (a[order_e] for a in (ecore, etile, edrel, eseg, eidx))

    NSEG = cfg.NSEG
    gid = ((ecore * TILES + etile) * NSEG + eseg).astype(np.int64)
    counts = np.bincount(gid, minlength=ncores * TILES * NSEG).reshape(ncores, TILES, NSEG)
    # chunks per (tile, seg): max over cores
    chunk_tbl = -(-counts.max(axis=0) // 128)  # [TILES, NSEG]

    # slot layout: blocks of BLOCK tiles; within block: seg-major; within
    # (block, seg): tiles in order, each (t,s) padded to chunk_tbl[t,s]*128
    blocks = [list(range(b, min(b + cfg.BLOCK, TILES))) for b in range(0, TILES, cfg.BLOCK)]
    regions = []       # (seg, slot_off, n_slots) -- one per (block, seg), idx-load granularity
    gathers = []       # (seg, slot_off, n_slots) -- <=1024-slot windows, dma_gather granularity
    GMAX = 1024
    ts_off = np.zeros((TILES, NSEG), dtype=np.int64)
    off = 0
    for blk in blocks:
        for s in range(NSEG):
            g_off = off
            for t in blk:
                ts_off[t, s] = off
                off += int(chunk_tbl[t, s]) * 128
            if off > g_off:
                regions.append((s, g_off, off - g_off))
                w = g_off
                while w < off:
                    n = min(GMAX, off - w)
                    gathers.append((s, w, n))
                    w += n
    TOT = off
    assert TOT % 128 == 0

    # place each core's edges into slots
    idx_all = np.zeros((ncores, TOT), dtype=np.int16)
    drel_all = np.full((ncores, TOT), -1.0, dtype=np.float32)
    # within-group position of each (sorted) edge
    grp_start = np.zeros(ncores * TILES * NSEG, dtype=np.int64)
    np.cumsum(counts.reshape(-1)[:-1], out=grp_start[1:])
    within = np.arange(len(gid)) - grp_start[gid]
    slot_of_edge = ts_off[etile, eseg] + within
    for c in range(ncores):
        m = ecore == c
        idx_all[c, slot_of_edge[m]] = eidx[m]
        drel_all[c, slot_of_edge[m]] = edrel[m].astype(np.float32)

    # wrapped layouts
    idx_wrapped = np.ascontiguousarray(
        np.tile(idx_all.reshape(ncores, TOT // 16, 16).transpose(0, 2, 1), (1, 8, 1))
    )  # [ncores, 128, TOT//16]
    drel_wrapped = np.ascontiguousarray(drel_all.reshape(ncores, TOT // 128, 128).transpose(0, 2, 1))
    # precomputed one-hot aggregation matrices: oh[c][p, chunk*128 + j] = 1
    # iff slot (chunk*128+p) has dst_rel == j
    import ml_dtypes
    oh_all = []
    for c in range(ncores):
        oh = np.zeros((128, TOT), dtype=ml_dtypes.bfloat16)
        slots = np.nonzero(drel_all[c] >= 0)[0]
        dr = drel_all[c][slots].astype(np.int64)
        oh[slots % 128, (slots // 128) * 128 + dr] = 1
        oh_all.append(oh)

    # per-core node-attribute wraps: [128, TILES]
    dinv_w = dinv_d.reshape(ncores, TILES, 128).transpose(0, 2, 1)
    batch_w = batch_d.reshape(ncores, TILES, 128).transpose(0, 2, 1)

    cnt = np.bincount(batch, minlength=G).astype(np.float32)
    cnt_inv = (1.0 / np.maximum(cnt, 1.0)).astype(np.float32)[:, None]  # [G,1]

    plan = dict(
        chunk_tbl=chunk_tbl, blocks=blocks, gathers=gathers, regions=regions,
        ts_off=ts_off, TOT=TOT,
    )
    percore = dict(
        x=[np.ascontiguousarray(x_d[c * NPC:(c + 1) * NPC]) for c in range(ncores)],
        idx=[np.ascontiguousarray(idx_wrapped[c]) for c in range(ncores)],
        drel=[np.ascontiguousarray(drel_wrapped[c]) for c in range(ncores)],
        oh=oh_all,
        dinv=[np.ascontiguousarray(dinv_w[c]) for c in range(ncores)],
        ndinv=[np.ascontiguousarray(-dinv_w[c]) for c in range(ncores)],
        batch=[np.ascontiguousarray(batch_w[c]) for c in range(ncores)],
    )
    return plan, percore, cnt_inv


# ---------------------------------------------------------------- program
def build_program(cfg, plan):
    TILES, NSEG, NPC = cfg.TILES, cfg.NSEG, cfg.NPC
    F, H, C, G = cfg.F, cfg.H, cfg.C, cfg.G
    chunk_tbl, blocks, gathers, regions, ts_off, TOT = (
        plan["chunk_tbl"], plan["blocks"], plan["gathers"], plan["regions"], plan["TOT"]
        if False else plan["ts_off"], plan["TOT"])
    chunk_tbl = plan["chunk_tbl"]; blocks = plan["blocks"]; gathers = plan["gathers"]
    regions = plan["regions"]; ts_off = plan["ts_off"]; TOT = plan["TOT"]

    nc = bacc.Bacc(num_devices=cfg.ncores, target_bir_lowering=False, num_swdge_queues=4)

    # ---- I/O -----------------------------------------------------------
    P = {}
    P["x"] = nc.declare_dram_parameter("x", [NPC, F], F32, isOutput=False)
    P["idx"] = nc.declare_dram_parameter("idx", [128, TOT // 16], I16, isOutput=False)
    P["oh"] = nc.declare_dram_parameter("oh", [128, TOT], BF16, isOutput=False)
    P["dinv"] = nc.declare_dram_parameter("dinv", [128, TILES], F32, isOutput=False)
    P["ndinv"] = nc.declare_dram_parameter("ndinv", [128, TILES], F32, isOutput=False)
    P["batch"] = nc.declare_dram_parameter("batch", [128, TILES], F32, isOutput=False)
    for l in range(3):
        P[f"Wa{l}"] = nc.declare_dram_parameter(f"Wa{l}", [F if l == 0 else H, H], F32, isOutput=False)
        P[f"Wb{l}"] = nc.declare_dram_parameter(f"Wb{l}", [F if l == 0 else H, H], F32, isOutput=False)
        P[f"bias{l}"] = nc.declare_dram_parameter(f"bias{l}", [1, H], F32, isOutput=False)
    P["Wlin"] = nc.declare_dram_parameter("Wlin", [H, C], F32, isOutput=False)
    P["blin"] = nc.declare_dram_parameter("blin", [1, C], F32, isOutput=False)
    P["cntinv"] = nc.declare_dram_parameter("cntinv", [G, 1], F32, isOutput=False)
    P["iota"] = nc.declare_dram_parameter("iota", [128, 128], F32, isOutput=False)
    P["ident"] = nc.declare_dram_parameter("ident", [128, 128], F32, isOutput=False)
    out_ext = nc.declare_dram_parameter("out", [G, C], F32, isOutput=True)

    # internal DRAM
    y_self = nc.dram_tensor("y_self", [NPC, 2 * H], BF16)
    y_full = nc.dram_tensor("y_full", [cfg.NPAD, 2 * H], BF16, addr_space="Shared")
    pool_in = nc.dram_tensor("pool_in", [G, H], F32)
    pool_out = nc.dram_tensor("pool_out", [G, H], F32, addr_space="Shared")

    CMAX = {s: 0 for s in range(NSEG)}   # max region cols per seg (in chunks)
    for (s, goff, n) in regions:
        CMAX[s] = max(CMAX[s], n // 128)

    with tile.TileContext(nc) as tc:
        with tc.tile_pool(name="const", bufs=1) as cpool, \
             tc.tile_pool(name="state", bufs=1) as spool, \
             tc.tile_pool(name="work", bufs=3) as wpool, \
             tc.tile_pool(name="msgs", bufs=3) as mpool, \
             tc.tile_pool(name="oh", bufs=4) as ohpool, \
             tc.tile_pool(name="psS", bufs=2, space="PSUM") as psS, \
             tc.tile_pool(name="psD", bufs=2, space="PSUM") as psD, \
             tc.tile_pool(name="psY", bufs=1, space="PSUM") as psY, \
             tc.tile_pool(name="psT", bufs=2, space="PSUM") as psT, \
             tc.tile_pool(name="psP", bufs=1, space="PSUM") as psP:

            # ---- load constants ----
            def cload(name, shape):
                t = cpool.tile(shape, F32, tag=name)
                nc.sync.dma_start(out=t[:], in_=P[name][:, :])
                return t

            iota_t = cload("iota", [128, 128])
            ident_t = cload("ident", [128, 128])
            dinv_t = cload("dinv", [128, TILES])
            ndinv_t = cload("ndinv", [128, TILES])
            batch_t = cload("batch", [128, TILES])

            cnt_t = cload("cntinv", [G, 1])
            Wa, Wb, bias = [], [], []
            for l in range(3):
                Wa.append(cload(f"Wa{l}", [F if l == 0 else H, H]))
                Wb.append(cload(f"Wb{l}", [F if l == 0 else H, H]))
                bias.append(cload(f"bias{l}", [1, H]))
            wlin_t = cload("Wlin", [H, C])
            blin_t = cload("blin", [1, C])
            ones_t = cpool.tile([1, 128], F32, tag="ones")
            nc.gpsimd.memset(ones_t[:], 1.0)
            # zero the pad halves of y_self rows once (never rewritten)
            zpad_t = cpool.tile([128, H], BF16, tag="zpad")
            nc.vector.memset(zpad_t[:], 0.0)
            for t in range(TILES):
                nc.sync.dma_start(out=y_self[t * 128:(t + 1) * 128, H:2 * H], in_=zpad_t[:])

            # persistent node state (h), one tag per tile
            h_tiles = [spool.tile([128, F], F32, tag=f"h{t}", name=f"h{t}") for t in range(TILES)]
            d_tiles = [spool.tile([128, H], F32, tag=f"d{t}", name=f"d{t}") for t in range(TILES)]

            psum_pool = psP.tile([G, H], F32, tag="pool")

            def prep_tile(l, t, h_in):
                """Per-tile dense prep for layer l: Y = dinv*(h@Wb[l]) -> y_self,
                d_tiles[t] = h@Wa[l] + bias[l]."""
                ps_t = psT.tile([F, 128], F32, tag="tr", name="ps_t")
                nc.tensor.transpose(ps_t[:], h_in[:], ident_t[:])
                hT = wpool.tile([F, 128], F32, tag="hT", name="hT")
                nc.vector.tensor_copy(hT[:], ps_t[:])
                ps_y = psY.tile([128, H], F32, tag="y", name="ps_y")
                nc.tensor.matmul(ps_y[:], hT[:], Wb[l][:], start=True, stop=True)
                y_sb = wpool.tile([128, H], BF16, tag="ysb", name="y_sb")
                nc.scalar.activation(y_sb[:], ps_y[:], mybir.ActivationFunctionType.Copy,
                                     scale=dinv_t[:, t:t + 1])
                nc.sync.dma_start(out=y_self[t * 128:(t + 1) * 128, 0:H], in_=y_sb[:])
                ps_d = psD.tile([128, H], F32, tag="d", name="ps_d")
                nc.tensor.matmul(ps_d[:], hT[:], Wa[l][:], start=True, stop=False)
                nc.tensor.matmul(ps_d[:], ones_t[:], bias[l][:], start=False, stop=True)
                nc.vector.tensor_copy(d_tiles[t][:], ps_d[:])

            def emit_ag():
                nc.gpsimd.collective_compute(
                    "AllGather", mybir.AluOpType.bypass,
                    replica_groups=[list(range(cfg.ncores))],
                    ins=[y_self[:, :].opt()], outs=[y_full[:, :].opt()],
                )

            # layer-0 prep from x, then first allgather
            for t in range(TILES):
                h_in = wpool.tile([128, F], F32, tag="xin")
                nc.sync.dma_start(out=h_in[:], in_=P["x"][t * 128:(t + 1) * 128, :])
                prep_tile(0, t, h_in)
            emit_ag()

            for l in range(3):
                # ---------- SpMM + combine (+ fused next-layer prep), per block ----------
                ri = 0
                wi = 0
                qn = 0
                for blk in blocks:
                    # region idx loads + window gathers for this block
                    blk_msgs = {}
                    for s in range(NSEG):
                        n_g = sum(int(chunk_tbl[t, s]) * 128 for t in blk)
                        if n_g == 0:
                            continue
                        (rs, roff, rn) = regions[ri]
                        assert rs == s and rn == n_g, (rs, s, rn, n_g, ri)
                        ri += 1
                        idx_t = wpool.tile([128, n_g // 16], I16, tag=f"idx{s}")
                        nc.sync.dma_start(out=idx_t[:],
                                          in_=P["idx"][:, roff // 16:(roff + n_g) // 16])
                        oh_t = mpool.tile([128, CMAX[s] * 128], BF16, tag=f"oh{s}")
                        nc.sync.dma_start(out=oh_t[:, :n_g],
                                          in_=P["oh"][:, roff:roff + n_g])
                        m_t = mpool.tile([128, CMAX[s], 2 * H], BF16, tag=f"m{s}")
                        w = roff
                        while w < roff + n_g:
                            (ws, woff, wn) = gathers[wi]
                            assert ws == s and woff == w, (ws, s, woff, w, wi)
                            wi += 1
                            nc.gpsimd.dma_gather(
                                m_t[:, (w - roff) // 128:(w - roff + wn) // 128, :],
                                y_full[s * cfg.SEGROWS:(s + 1) * cfg.SEGROWS, :],
                                idx_t[:, (w - roff) // 16:(w - roff + wn) // 16],
                                wn, wn, 2 * H, queue_num=qn)
                            qn = (qn + 1) % 4
                            w += wn
                        blk_msgs[s] = (m_t, oh_t, roff)

                    for t in blk:
                        nch = int(chunk_tbl[t].sum())
                        ps_s = None
                        if nch > 0:
                            ps_s = psS.tile([128, H], F32, tag="s")
                            ci = 0
                            for s in range(NSEG):
                                nck = int(chunk_tbl[t, s])
                                if nck == 0:
                                    continue
                                m_t, oh_t, roff2 = blk_msgs[s]
                                lo = (int(ts_off[t, s]) - roff2) // 128
                                for c in range(nck):
                                    nc.tensor.matmul(
                                        ps_s[:], oh_t[:, (lo + c) * 128:(lo + c + 1) * 128],
                                        m_t[:, lo + c, 0:H],
                                        start=(ci == 0), stop=(ci == nch - 1))
                                    ci += 1
                        # combine: h_next = (relu?)(d + (-dinv)*ps_s)
                        if l < 2:
                            if nch > 0:
                                tmp2 = wpool.tile([128, H], F32, tag="cmb2")
                                nc.vector.scalar_tensor_tensor(
                                    out=tmp2[:], in0=ps_s[:], scalar=ndinv_t[:, t:t + 1],
                                    in1=d_tiles[t][:], op0=mybir.AluOpType.mult,
                                    op1=mybir.AluOpType.add)
                            else:
                                tmp2 = d_tiles[t]
                            nc.scalar.activation(h_tiles[t][:], tmp2[:],
                                                 mybir.ActivationFunctionType.Relu)
                            prep_tile(l + 1, t, h_tiles[t])
                        else:
                            if nch > 0:
                                h3 = wpool.tile([128, H], F32, tag="h3")
                                nc.vector.scalar_tensor_tensor(
                                    out=h3[:], in0=ps_s[:], scalar=ndinv_t[:, t:t + 1],
                                    in1=d_tiles[t][:], op0=mybir.AluOpType.mult,
                                    op1=mybir.AluOpType.add)
                            else:
                                h3 = d_tiles[t]
                            # pooling: psum_pool += onehot(batch)^T @ h3
                            poh = ohpool.tile([128, G], F32, tag="poh")
                            nc.vector.tensor_scalar(
                                out=poh[:], in0=iota_t[:, :G],
                                scalar1=batch_t[:, t:t + 1],
                                scalar2=None, op0=mybir.AluOpType.is_equal)
                            nc.tensor.matmul(psum_pool[:], poh[:], h3[:],
                                             start=(t == 0), stop=(t == TILES - 1),
                                             skip_group_check=True)
                assert ri == len(regions) and wi == len(gathers)
                if l < 2:
                    emit_ag()

            # ---------- pooling: allreduce, scale, final linear ----------
            pool_sb = wpool.tile([G, H], F32, tag="poolsb")
            nc.vector.tensor_copy(pool_sb[:], psum_pool[:])
            nc.sync.dma_start(out=pool_in[:, :], in_=pool_sb[:])
            nc.gpsimd.collective_compute(
                "AllReduce", mybir.AluOpType.add,
                replica_groups=[list(range(cfg.ncores))],
                ins=[pool_in[:, :].opt()], outs=[pool_out[:, :].opt()],
            )
            pool_g = wpool.tile([G, H], F32, tag="poolg")
            nc.sync.dma_start(out=pool_g[:], in_=pool_out[:, :])
            pooled = wpool.tile([G, H], F32, tag="pooled")
            nc.vector.tensor_scalar(out=pooled[:], in0=pool_g[:], scalar1=cnt_t[:, 0:1],
                                    scalar2=None, op0=mybir.AluOpType.mult)
            # transpose pooled -> [H, G]
            ps_pt = psT.tile([H, G], F32, tag="tr")
            nc.tensor.transpose(ps_pt[:], pooled[:], ident_t[:G, :G])
            pooledT = wpool.tile([H, G], F32, tag="pooledT")
            nc.vector.tensor_copy(pooledT[:], ps_pt[:])
            ps_o = psY.tile([G, C], F32, tag="y")
            nc.tensor.matmul(ps_o[:], pooledT[:], wlin_t[:], start=True, stop=False)
            nc.tensor.matmul(ps_o[:], ones_t[:, :G], blin_t[:], start=False, stop=True)
            out_sb = wpool.tile([G, C], F32, tag="outsb")
            nc.vector.tensor_copy(out_sb[:], ps_o[:])
            nc.sync.dma_start(out=out_ext[:, :], in_=out_sb[:])

    nc.compile()
    return nc


# ---------------------------------------------------------------- driver
def make_in_maps(cfg, percore, cnt_inv, W1, b1, W2, b2, W3, b3, Wlin, blin):
    iota = np.tile(np.arange(128, dtype=np.float32)[None, :], (128, 1))
    ident = np.eye(128, dtype=np.float32)
    Ws = [np.asarray(W1, np.float32), np.asarray(W2, np.float32), np.asarray(W3, np.float32)]
    bs = [np.asarray(b1, np.float32), np.asarray(b2, np.float32), np.asarray(b3, np.float32)]
    in_maps = []
    for c in range(cfg.ncores):
        m = {
            "x": percore["x"][c],
            "idx": percore["idx"][c],
            "oh": percore["oh"][c],
            "dinv": percore["dinv"][c],
            "ndinv": percore["ndinv"][c],
            "batch": percore["batch"][c],
            "cntinv": cnt_inv,
            "iota": iota,
            "ident": ident,
            "Wlin": np.ascontiguousarray(Wlin, dtype=np.float32),
            "blin": np.ascontiguousarray(blin, dtype=np.float32)[None, :],
        }
        for l in range(3):
            m[f"Wa{l}"] = np.ascontiguousarray(Ws[l][0])
            m[f"Wb{l}"] = np.ascontiguousarray(Ws[l][1])
            m[f"bias{l}"] = np.ascontiguousarray(bs[l])[None, :]
        in_maps.append(m)
    return in_maps


def run(cfg, inputs, trace=False):
    plan, percore, cnt_inv = host_prep(cfg, inputs["x"], inputs["edge_index"], inputs["batch"])
    nc = build_program(cfg, plan)
    in_maps = make_in_maps(cfg, percore, cnt_inv,
                           inputs["W1"], inputs["b1"], inputs["W2"], inputs["b2"],
                           inputs["W3"], inputs["b3"], inputs["Wlin"], inputs["blin"])
    res = run_bass_kernel_spmd(nc, in_maps, core_ids=list(range(cfg.ncores)), trace=trace)
    return np.asarray(res.results[0]["out"]), res


def kernel(**inputs) -> np.ndarray:
    out, _ = run(FULL, inputs, trace=False)
    return out



# revision 11
# speedup vs baseline: 1.3934x; 1.3934x over previous
"""ChebNet (K=2, 3 layers + global mean pool + linear) on 8 Trainium2 NeuronCores.

Strategy (pull-based graph parallel, v2):
  - Nodes dealt (in-degree balanced) across 8 cores x 98 tiles of 128.
  - Node state kept TRANSPOSED in SBUF (hT [64,128] bf16 per tile).
  - Layers 1,2 are real SpMMs: per layer, y = dinv*(h@Wb) is written per
    AG *piece* (4 row-pieces of ~25 tiles); each piece AllGathers as soon
    as its tiles are prepped, overlapping collectives with compute.
    Messages dma_gather'd (one gather per (4-tile block, piece), 256B rows),
    one-hot built on DVE per 128-slot chunk via is_equal(iota, dst_rel)
    scaled by -dinv[dst] (folds the normalization+sign into the matmul),
    accumulated into PSUM on top of h@Wa + bias, then ReLU.
  - Layer 3 is algebraically eliminated: pooling is linear, so
    sum_{n in g} (-A_hat h2 @ W3b) = (Wp^T z2) with z2 = dinv*(h2@W3b) and
    Wp[n,g] = -cntinv[g] * sum_{e: src=n, batch[dst]=g} dinv[dst] computed
    host-side from graph structure only. Remaining terms pool via
    one-hot(batch)*cntinv matmuls. One [64,64] AllReduce + tiny linear.
"""
import sys

for _p in ("/opt/trn_rl_repo",):
    if _p not in sys.path:
        sys.path.insert(0, _p)

import numpy as np
import ml_dtypes
import concourse.bass as bass
import concourse.mybir as mybir
from concourse import bacc, tile
from concourse.bass_utils import run_bass_kernel_spmd

F32 = mybir.dt.float32
BF16 = mybir.dt.bfloat16
I16 = mybir.dt.int16


class Cfg:
    def __init__(self, N, E, F, H, C, G, ncores=8, block=4):
        self.N, self.E, self.F, self.H, self.C, self.G = N, E, F, H, C, G
        self.ncores = ncores
        npc = -(-N // (ncores * 128)) * 128
        self.NPC = npc
        self.NPAD = npc * ncores
        self.TILES = npc // 128          # 98
        self.BLOCK = block
        # AG pieces: tile ranges per piece (4 pieces)
        base = self.TILES // 4
        extra = self.TILES % 4
        nts = [base + (1 if i < extra else 0) for i in range(4)]
        self.PIECE_NT = nts              # [25, 25, 24, 24]
        self.PIECE_T0 = [sum(nts[:i]) for i in range(4)]
        self.PIECE_ROWS = [nt * 128 for nt in nts]
        self.SEGROWS = [ncores * r for r in self.PIECE_ROWS]
        assert all(s <= 32767 for s in self.SEGROWS)
        self.NSEG = 4


FULL = Cfg(N=100000, E=1600000, F=64, H=64, C=16, G=64)


# ---------------------------------------------------------------- host prep
def host_prep(cfg, x, edge_index, batch):
    N, G = cfg.N, cfg.G
    ncores, TILES, NPC = cfg.ncores, cfg.TILES, cfg.NPC
    NSEG = cfg.NSEG
    src = np.asarray(edge_index[0], dtype=np.int64)
    dst = np.asarray(edge_index[1], dtype=np.int64)
    batch = np.asarray(batch, dtype=np.int64)

    deg = np.bincount(src, minlength=N).astype(np.float64)
    dinv = np.where(deg > 0, 1.0 / np.sqrt(np.maximum(deg, 1.0)), 0.0).astype(np.float32)

    # ---- deal nodes into (core, tile) bins, balancing in-degree ----
    indeg = np.bincount(dst, minlength=N)
    order = np.argsort(-indeg, kind="stable")
    nbins = ncores * TILES
    k = np.arange(N)
    rnd = k // nbins
    pos = k % nbins
    binid = np.where(rnd % 2 == 0, pos, nbins - 1 - pos)
    core_of_bin = binid % ncores
    tile_of_bin = binid // ncores
    g_of_sorted = core_of_bin * NPC + tile_of_bin * 128 + rnd
    dealt = np.empty(N, dtype=np.int64)
    dealt[order] = g_of_sorted

    src_g = dealt[src]
    dst_g = dealt[dst]

    # per-node (dealt) attributes
    dinv_d = np.zeros(cfg.NPAD, dtype=np.float32)
    dinv_d[dealt] = dinv
    batch_d = np.full(cfg.NPAD, -1.0, dtype=np.float32)
    batch_d[dealt] = batch.astype(np.float32)
    x_d = np.zeros((cfg.NPAD, cfg.F), dtype=np.float32)
    x_d[dealt] = np.asarray(x, dtype=np.float32)

    cnt = np.bincount(batch, minlength=G).astype(np.float32)
    cinv = np.where(cnt > 0, 1.0 / np.maximum(cnt, 1.0), 0.0).astype(np.float32)
    cnt01 = (cnt > 0).astype(np.float32)
    cinv_d = np.zeros(cfg.NPAD, dtype=np.float32)
    bidx = batch_d.astype(np.int64)
    cinv_d[bidx >= 0] = cinv[bidx[bidx >= 0]]

    # ---- edge organization: (dst core, dst tile, src piece) ----
    e_core = dst_g // NPC
    e_tile = (dst_g % NPC) // 128
    e_drel = dst_g % 128
    s_tile = (src_g % NPC) // 128
    s_core = src_g // NPC
    s_slot = src_g % 128
    t0s = np.array(cfg.PIECE_T0)
    e_seg = np.searchsorted(t0s, s_tile, side="right") - 1
    rows_p = np.array(cfg.PIECE_ROWS)[e_seg]
    e_idx = s_core * rows_p + (s_tile - t0s[e_seg]) * 128 + s_slot

    order_e = np.lexsort((src_g, e_seg, e_tile, e_core))
    e_core, e_tile, e_drel, e_seg, e_idx = (a[order_e] for a in
                                            (e_core, e_tile, e_drel, e_seg, e_idx))
    dst_go = dst_g[order_e]

    gid = ((e_core * TILES + e_tile) * NSEG + e_seg).astype(np.int64)
    counts = np.bincount(gid, minlength=ncores * TILES * NSEG).reshape(ncores, TILES, NSEG)
    chunk_tbl = -(-counts.max(axis=0) // 128)  # [TILES, NSEG]

    blocks = [list(range(b, min(b + cfg.BLOCK, TILES))) for b in range(0, TILES, cfg.BLOCK)]
    regions = []       # (seg, slot_off, n_slots) -- one per (block, seg), one gather each
    ts_off = np.zeros((TILES, NSEG), dtype=np.int64)
    off = 0
    for blk in blocks:
        for s in range(NSEG):
            g_off = off
            for t in blk:
                ts_off[t, s] = off
                off += int(chunk_tbl[t, s]) * 128
            if off > g_off:
                regions.append((s, g_off, off - g_off))
    TOT = off
    assert TOT % 128 == 0

    # place each core's edges into slots
    idx_all = np.zeros((ncores, TOT), dtype=np.int16)
    drel_all = np.full((ncores, TOT), -1.0, dtype=np.float32)
    ndv_all = np.zeros((ncores, TOT), dtype=np.float32)
    grp_start = np.zeros(ncores * TILES * NSEG, dtype=np.int64)
    np.cumsum(counts.reshape(-1)[:-1], out=grp_start[1:])
    within = np.arange(len(gid)) - grp_start[gid]
    slot_of_edge = ts_off[e_tile, e_seg] + within
    ndv_e = -dinv_d[dst_go]
    for c in range(ncores):
        m = e_core == c
        idx_all[c, slot_of_edge[m]] = e_idx[m].astype(np.int16)
        drel_all[c, slot_of_edge[m]] = e_drel[m].astype(np.float32)
        ndv_all[c, slot_of_edge[m]] = ndv_e[m]

    # wrapped layouts
    idx_wrapped = np.ascontiguousarray(
        np.tile(idx_all.reshape(ncores, TOT // 16, 16).transpose(0, 2, 1), (1, 8, 1))
    )  # [ncores, 128, TOT//16]
    drel_w = drel_all.reshape(ncores, TOT // 128, 128).transpose(0, 2, 1)
    ndv_w = ndv_all.reshape(ncores, TOT // 128, 128).transpose(0, 2, 1)

    # pool-weight matrix (layer-3 elimination), rows = dealt node ids
    Wp = np.zeros((cfg.NPAD, G), np.float32)
    np.add.at(Wp, (src_g, batch[dst]), -dinv[dst])
    Wp *= cinv[None, :]

    # per-core node-attribute wraps: [128, TILES]
    dinv_wt = dinv_d.reshape(ncores, TILES, 128).transpose(0, 2, 1)
    batch_wt = batch_d.reshape(ncores, TILES, 128).transpose(0, 2, 1)
    cinv_wt = cinv_d.reshape(ncores, TILES, 128).transpose(0, 2, 1)

    plan = dict(chunk_tbl=chunk_tbl, blocks=blocks, regions=regions,
                ts_off=ts_off, TOT=TOT)
    percore = dict(
        x=[np.ascontiguousarray(x_d[c * NPC:(c + 1) * NPC]) for c in range(ncores)],
        idx=[np.ascontiguousarray(idx_wrapped[c]) for c in range(ncores)],
        drel=[np.ascontiguousarray(drel_w[c]) for c in range(ncores)],
        ndv=[np.ascontiguousarray(ndv_w[c]) for c in range(ncores)],
        Wp=[np.ascontiguousarray(Wp[c * NPC:(c + 1) * NPC]).astype(ml_dtypes.bfloat16)
            for c in range(ncores)],
        dinv=[np.ascontiguousarray(dinv_wt[c]) for c in range(ncores)],
        batch=[np.ascontiguousarray(batch_wt[c]) for c in range(ncores)],
        cinv=[np.ascontiguousarray(cinv_wt[c]) for c in range(ncores)],
    )
    return plan, percore, cnt01


# ---------------------------------------------------------------- program
def build_program(cfg, plan):
    TILES, NSEG, NPC = cfg.TILES, cfg.NSEG, cfg.NPC
    F, H, C, G = cfg.F, cfg.H, cfg.C, cfg.G
    chunk_tbl = plan["chunk_tbl"]; blocks = plan["blocks"]
    regions = plan["regions"]; ts_off = plan["ts_off"]; TOT = plan["TOT"]
    P_T0, P_NT = cfg.PIECE_T0, cfg.PIECE_NT
    PIECE_ROWS, SEGROWS = cfg.PIECE_ROWS, cfg.SEGROWS
    piece_of_tile = np.searchsorted(np.array(P_T0), np.arange(TILES), side="right") - 1
    piece_end_tile = [P_T0[p] + P_NT[p] - 1 for p in range(4)]

    # max chunks per (block,seg) region -> fixed msg tile shapes
    CMAXB = {s: 1 for s in range(NSEG)}
    for (s, goff, n) in regions:
        CMAXB[s] = max(CMAXB[s], n // 128)

    nc = bacc.Bacc(num_devices=cfg.ncores, target_bir_lowering=False, num_swdge_queues=4)

    # ---- I/O -----------------------------------------------------------
    P = {}
    P["x"] = nc.declare_dram_parameter("x", [NPC, F], BF16, isOutput=False)
    P["idx"] = nc.declare_dram_parameter("idx", [128, TOT // 16], I16, isOutput=False)
    P["drel"] = nc.declare_dram_parameter("drel", [128, TOT // 128], F32, isOutput=False)
    P["ndv"] = nc.declare_dram_parameter("ndv", [128, TOT // 128], F32, isOutput=False)
    P["Wp"] = nc.declare_dram_parameter("Wp", [NPC, G], BF16, isOutput=False)
    P["dinv"] = nc.declare_dram_parameter("dinv", [128, TILES], F32, isOutput=False)
    P["batch"] = nc.declare_dram_parameter("batch", [128, TILES], F32, isOutput=False)
    P["cinv"] = nc.declare_dram_parameter("cinv", [128, TILES], F32, isOutput=False)
    for l in range(3):
        P[f"Wa{l}"] = nc.declare_dram_parameter(f"Wa{l}", [F if l == 0 else H, H], BF16, isOutput=False)
        P[f"Wb{l}"] = nc.declare_dram_parameter(f"Wb{l}", [F if l == 0 else H, H], BF16, isOutput=False)
        P[f"bias{l}"] = nc.declare_dram_parameter(f"bias{l}", [1, H], BF16, isOutput=False)
    P["Wlin"] = nc.declare_dram_parameter("Wlin", [H, C], F32, isOutput=False)
    P["blin"] = nc.declare_dram_parameter("blin", [1, C], F32, isOutput=False)
    P["cnt01"] = nc.declare_dram_parameter("cnt01", [1, G], BF16, isOutput=False)
    P["iota"] = nc.declare_dram_parameter("iota", [128, 128], BF16, isOutput=False)
    P["identb"] = nc.declare_dram_parameter("identb", [128, 128], BF16, isOutput=False)
    out_ext = nc.declare_dram_parameter("out", [G, C], F32, isOutput=True)

    # internal DRAM: per-piece AG in/out (out double-buffered per layer)
    y_self = [nc.dram_tensor(f"y_self{p}", [PIECE_ROWS[p], 2 * H], BF16)
              for p in range(4)]
    y_piece = [[nc.dram_tensor(f"y_piece{li}_{p}", [SEGROWS[p], 2 * H], BF16,
                               addr_space="Shared") for p in range(4)]
               for li in range(2)]
    pool_in = nc.dram_tensor("pool_in", [H, G], F32)
    pool_out = nc.dram_tensor("pool_out", [H, G], F32, addr_space="Shared")

    with tile.TileContext(nc) as tc:
        with tc.tile_pool(name="const", bufs=1) as cpool, \
             tc.tile_pool(name="state", bufs=1) as spool, \
             tc.tile_pool(name="work", bufs=3) as wpool, \
             tc.tile_pool(name="msgs", bufs=2) as mpool, \
             tc.tile_pool(name="oh", bufs=6) as ohpool, \
             tc.tile_pool(name="psS", bufs=2, space="PSUM") as psS, \
             tc.tile_pool(name="psT", bufs=2, space="PSUM") as psT, \
             tc.tile_pool(name="psY", bufs=2, space="PSUM") as psY, \
             tc.tile_pool(name="psPZ", bufs=1, space="PSUM") as psPZ, \
             tc.tile_pool(name="psPH", bufs=1, space="PSUM") as psPH:

            # ---- load constants ----
            def cload(name, shape, dt=F32):
                t = cpool.tile(shape, dt, tag=name)
                nc.sync.dma_start(out=t[:], in_=P[name][:, :])
                return t

            iota_t = cload("iota", [128, 128], BF16)
            identb_t = cload("identb", [128, 128], BF16)
            dinv_t = cload("dinv", [128, TILES])
            batch_t = cload("batch", [128, TILES])
            cinv_t = cload("cinv", [128, TILES])
            drel_t = cload("drel", [128, TOT // 128])
            ndv_t = cload("ndv", [128, TOT // 128])
            idx_t = cpool.tile([128, TOT // 16], I16, tag="idx")
            nc.sync.dma_start(out=idx_t[:], in_=P["idx"][:, :])
            cnt01_t = cload("cnt01", [1, G], BF16)
            Wa, Wb, bias = [], [], []
            for l in range(3):
                Wa.append(cload(f"Wa{l}", [F if l == 0 else H, H], BF16))
                Wb.append(cload(f"Wb{l}", [F if l == 0 else H, H], BF16))
                bias.append(cload(f"bias{l}", [1, H], BF16))
            wlin_t = cload("Wlin", [H, C])
            blin_t = cload("blin", [1, C])
            onesb_t = cpool.tile([1, 128], BF16, tag="onesb")
            nc.gpsimd.memset(onesb_t[:], 1.0)
            ones_t = cpool.tile([1, 128], F32, tag="ones")
            nc.gpsimd.memset(ones_t[:], 1.0)
            # zero the pad halves of y_self rows once
            zpad_t = cpool.tile([128, H], BF16, tag="zpad")
            nc.vector.memset(zpad_t[:], 0.0)
            for p in range(4):
                for tt in range(P_NT[p]):
                    nc.sync.dma_start(out=y_self[p][tt * 128:(tt + 1) * 128, H:2 * H],
                                      in_=zpad_t[:])

            # persistent transposed node state, two layer slots
            hT = [[spool.tile([F, 128], BF16, tag=f"hT{a}_{t}", name=f"hT{a}_{t}")
                   for t in range(TILES)] for a in range(2)]

            psum_pz = psPZ.tile([H, G], F32, tag="pz")
            psum_ph = psPH.tile([H, G], F32, tag="ph")

            def y_prep(l, t, hT_t):
                """y = dinv*(h@Wb[l]) for tile t -> y_self piece; AG when piece done."""
                ps_y = psY.tile([128, H], F32, tag="y", name="ps_y")
                nc.tensor.matmul(ps_y[:], hT_t[:], Wb[l][:], start=True, stop=True)
                y_sb = wpool.tile([128, H], BF16, tag="ysb", name="y_sb")
                nc.scalar.activation(y_sb[:], ps_y[:], mybir.ActivationFunctionType.Copy,
                                     scale=dinv_t[:, t:t + 1])
                p = int(piece_of_tile[t])
                tt = t - P_T0[p]
                nc.sync.dma_start(out=y_self[p][tt * 128:(tt + 1) * 128, 0:H], in_=y_sb[:])
                li = l  # y for SpMM layer l reads buffer set l
                if t == piece_end_tile[p]:
                    nc.gpsimd.collective_compute(
                        "AllGather", mybir.AluOpType.bypass,
                        replica_groups=[list(range(cfg.ncores))],
                        ins=[y_self[p][:, :].opt()], outs=[y_piece[li][p][:, :].opt()],
                    )

            # ---------- L0 prep: x -> hT[0], y1 pieces ----------
            for t in range(TILES):
                x_in = wpool.tile([128, F], BF16, tag="xin")
                nc.sync.dma_start(out=x_in[:], in_=P["x"][t * 128:(t + 1) * 128, :])
                ps_t = psT.tile([F, 128], BF16, tag="tr", name="ps_t")
                nc.tensor.transpose(ps_t[:], x_in[:], identb_t[:])
                nc.vector.tensor_copy(hT[0][t][:], ps_t[:])
                y_prep(0, t, hT[0][t])

            # ---------- SpMM layers (li = 0, 1) ----------
            for li in range(2):
                ri = 0
                qn = 0
                for blk in blocks:
                    blk_msgs = {}
                    for s in range(NSEG):
                        n_g = sum(int(chunk_tbl[t, s]) * 128 for t in blk)
                        if n_g == 0:
                            continue
                        (rs, roff, rn) = regions[ri]
                        assert rs == s and rn == n_g, (rs, s, rn, n_g, ri)
                        ri += 1
                        m_t = mpool.tile([128, CMAXB[s], 2 * H], BF16, tag=f"m{s}")
                        # split into <=1024-slot windows (descriptor carveout:
                        # dynamic_dma_scratch_size // 16 = 1024 descs per queue)
                        nck_r = n_g // 128
                        nwin = -(-nck_r // 8)
                        base = nck_r // nwin
                        rem = nck_r % nwin
                        w0 = 0
                        for wi in range(nwin):
                            wc = base + (1 if wi < rem else 0)
                            wn = wc * 128
                            woff = roff + w0 * 128
                            nc.gpsimd.dma_gather(
                                m_t[:, w0:w0 + wc, :],
                                y_piece[li][s][0:SEGROWS[s], :],
                                idx_t[:, woff // 16:(woff + wn) // 16],
                                wn, wn, 2 * H, queue_num=qn)
                            qn = (qn + 1) % 4
                            w0 += wc
                        blk_msgs[s] = (m_t, roff)

                    for t in blk:
                        ps_s = psS.tile([128, H], F32, tag="s")
                        nc.tensor.matmul(ps_s[:], hT[li][t][:], Wa[li][:],
                                         start=True, stop=False)
                        nc.tensor.matmul(ps_s[:], onesb_t[:], bias[li][:],
                                         start=False, stop=False)
                        nch = int(chunk_tbl[t].sum())
                        ci = 0
                        for s in range(NSEG):
                            nck = int(chunk_tbl[t, s])
                            if nck == 0:
                                continue
                            m_t, roff2 = blk_msgs[s]
                            lo = (int(ts_off[t, s]) - roff2) // 128
                            for ck in range(nck):
                                col = (int(ts_off[t, s]) + ck * 128) // 128
                                oh_t = ohpool.tile([128, 128], BF16, tag="oh")
                                nc.vector.tensor_scalar(
                                    out=oh_t[:], in0=iota_t[:],
                                    scalar1=drel_t[:, col:col + 1],
                                    scalar2=ndv_t[:, col:col + 1],
                                    op0=mybir.AluOpType.is_equal,
                                    op1=mybir.AluOpType.mult)
                                ci += 1
                                nc.tensor.matmul(
                                    ps_s[:], oh_t[:], m_t[:, lo + ck, 0:H],
                                    start=False, stop=(ci == nch))
                        h_sb = wpool.tile([128, H], BF16, tag="hsb")
                        nc.scalar.activation(h_sb[:], ps_s[:],
                                             mybir.ActivationFunctionType.Relu)
                        ps_t = psT.tile([H, 128], BF16, tag="tr", name="ps_t")
                        nc.tensor.transpose(ps_t[:], h_sb[:], identb_t[:])
                        if li == 0:
                            nc.vector.tensor_copy(hT[1][t][:], ps_t[:])
                            y_prep(li + 1, t, hT[1][t])
                        else:
                            # L2 tail: z = dinv*(h2@W3b); pool accumulations
                            hTc = wpool.tile([H, 128], BF16, tag="hTc")
                            nc.vector.tensor_copy(hTc[:], ps_t[:])
                            ps_y = psY.tile([128, H], F32, tag="y", name="ps_y")
                            nc.tensor.matmul(ps_y[:], hTc[:], Wb[2][:],
                                             start=True, stop=True)
                            z_sb = wpool.tile([128, H], BF16, tag="zsb")
                            nc.scalar.activation(z_sb[:], ps_y[:],
                                                 mybir.ActivationFunctionType.Copy,
                                                 scale=dinv_t[:, t:t + 1])
                            wp_t = wpool.tile([128, G], BF16, tag="wp")
                            nc.sync.dma_start(out=wp_t[:],
                                              in_=P["Wp"][t * 128:(t + 1) * 128, :])
                            poh_t = ohpool.tile([128, G], BF16, tag="poh")
                            nc.vector.tensor_scalar(
                                out=poh_t[:], in0=iota_t[:, :G],
                                scalar1=batch_t[:, t:t + 1],
                                scalar2=cinv_t[:, t:t + 1],
                                op0=mybir.AluOpType.is_equal,
                                op1=mybir.AluOpType.mult)
                            nc.tensor.matmul(psum_ph[:], h_sb[:], poh_t[:],
                                             start=(t == 0), stop=(t == TILES - 1),
                                             skip_group_check=True)
                            nc.tensor.matmul(psum_pz[:], z_sb[:], wp_t[:],
                                             start=(t == 0), stop=False,
                                             skip_group_check=True)
                assert ri == len(regions), (ri, len(regions))

            # ---------- finish pooled^T = psum_pz + W3a^T @ P2T + b3 x cnt01 ----------
            p2t_sb = wpool.tile([H, G], BF16, tag="p2t")
            nc.vector.tensor_copy(p2t_sb[:], psum_ph[:])
            nc.tensor.matmul(psum_pz[:], Wa[2][:], p2t_sb[:],
                             start=False, stop=False, skip_group_check=True)
            nc.tensor.matmul(psum_pz[:], bias[2][:], cnt01_t[:],
                             start=False, stop=True, skip_group_check=True)
            pl_sb = wpool.tile([H, G], F32, tag="plsb")
            nc.vector.tensor_copy(pl_sb[:], psum_pz[:])
            nc.sync.dma_start(out=pool_in[:, :], in_=pl_sb[:])
            nc.gpsimd.collective_compute(
                "AllReduce", mybir.AluOpType.add,
                replica_groups=[list(range(cfg.ncores))],
                ins=[pool_in[:, :].opt()], outs=[pool_out[:, :].opt()],
            )
            arT = wpool.tile([H, G], F32, tag="arT")
            nc.sync.dma_start(out=arT[:], in_=pool_out[:, :])
            ps_yo = psY.tile([128, H], F32, tag="y", name="ps_y")
            ps_o = ps_yo[0:G, 0:C]
            nc.tensor.matmul(ps_o, arT[:], wlin_t[:], start=True, stop=False)
            nc.tensor.matmul(ps_o, ones_t[:, :G], blin_t[:], start=False, stop=True)
            out_sb = wpool.tile([G, C], F32, tag="outsb")
            nc.vector.tensor_copy(out_sb[:], ps_o)
            nc.sync.dma_start(out=out_ext[:, :], in_=out_sb[:])

    nc.compile()
    return nc


# ---------------------------------------------------------------- driver
def make_in_maps(cfg, percore, cnt01, W1, b1, W2, b2, W3, b3, Wlin, blin):
    iota = np.tile(np.arange(128, dtype=np.float32)[None, :], (128, 1))
    ident = np.eye(128, dtype=np.float32)  # identb only
    Ws = [np.asarray(W1, np.float32), np.asarray(W2, np.float32), np.asarray(W3, np.float32)]
    bs = [np.asarray(b1, np.float32), np.asarray(b2, np.float32), np.asarray(b3, np.float32)]
    bf = ml_dtypes.bfloat16
    in_maps = []
    for c in range(cfg.ncores):
        m = {
            "x": percore["x"][c].astype(bf),
            "idx": percore["idx"][c],
            "drel": percore["drel"][c],
            "ndv": percore["ndv"][c],
            "Wp": percore["Wp"][c],
            "dinv": percore["dinv"][c],
            "batch": percore["batch"][c],
            "cinv": percore["cinv"][c],
            "cnt01": cnt01[None, :].astype(bf),
            "iota": iota.astype(bf),
            "identb": ident.astype(bf),
            "Wlin": np.ascontiguousarray(Wlin, dtype=np.float32),
            "blin": np.ascontiguousarray(blin, dtype=np.float32)[None, :],
        }
        for l in range(3):
            m[f"Wa{l}"] = np.ascontiguousarray(Ws[l][0]).astype(bf)
            m[f"Wb{l}"] = np.ascontiguousarray(Ws[l][1]).astype(bf)
            m[f"bias{l}"] = np.ascontiguousarray(bs[l])[None, :].astype(bf)
        in_maps.append(m)
    return in_maps


def run(cfg, inputs, trace=False):
    plan, percore, cnt01 = host_prep(cfg, inputs["x"], inputs["edge_index"], inputs["batch"])
    nc = build_program(cfg, plan)
    in_maps = make_in_maps(cfg, percore, cnt01,
                           inputs["W1"], inputs["b1"], inputs["W2"], inputs["b2"],
                           inputs["W3"], inputs["b3"], inputs["Wlin"], inputs["blin"])
    res = run_bass_kernel_spmd(nc, in_maps, core_ids=list(range(cfg.ncores)), trace=trace)
    return np.asarray(res.results[0]["out"]), res


def kernel(**inputs) -> np.ndarray:
    out, _ = run(FULL, inputs, trace=False)
    return out


# revision 14
# speedup vs baseline: 1.7167x; 1.2320x over previous
"""ChebNet (K=2, 3 layers + global mean pool + linear) on 8 Trainium2 NeuronCores.

Strategy (pull-based graph parallel, v2):
  - Nodes dealt (in-degree balanced) across 8 cores x 98 tiles of 128.
  - Node state kept TRANSPOSED in SBUF (hT [64,128] bf16 per tile).
  - Layers 1,2 are real SpMMs: per layer, y = dinv*(h@Wb) is written per
    AG *piece* (4 row-pieces of ~25 tiles); each piece AllGathers as soon
    as its tiles are prepped, overlapping collectives with compute.
    Messages dma_gather'd (one gather per (4-tile block, piece), 256B rows),
    one-hot built on DVE per 128-slot chunk via is_equal(iota, dst_rel)
    scaled by -dinv[dst] (folds the normalization+sign into the matmul),
    accumulated into PSUM on top of h@Wa + bias, then ReLU.
  - Layer 3 is algebraically eliminated: pooling is linear, so
    sum_{n in g} (-A_hat h2 @ W3b) = (Wp^T z2) with z2 = dinv*(h2@W3b) and
    Wp[n,g] = -cntinv[g] * sum_{e: src=n, batch[dst]=g} dinv[dst] computed
    host-side from graph structure only. Remaining terms pool via
    one-hot(batch)*cntinv matmuls. One [64,64] AllReduce + tiny linear.
"""
import sys

for _p in ("/opt/trn_rl_repo",):
    if _p not in sys.path:
        sys.path.insert(0, _p)

import numpy as np
import ml_dtypes
import concourse.bass as bass
import concourse.mybir as mybir
from concourse import bacc, tile
from concourse.bass_utils import run_bass_kernel_spmd

F32 = mybir.dt.float32
BF16 = mybir.dt.bfloat16
I16 = mybir.dt.int16


class Cfg:
    def __init__(self, N, E, F, H, C, G, ncores=8, block=4):
        self.N, self.E, self.F, self.H, self.C, self.G = N, E, F, H, C, G
        self.ncores = ncores
        npc = -(-N // (ncores * 128)) * 128
        self.NPC = npc
        self.NPAD = npc * ncores
        self.TILES = npc // 128          # 98
        self.BLOCK = block
        # AG pieces: tile ranges per piece (4 pieces)
        base = self.TILES // 4
        extra = self.TILES % 4
        nts = [base + (1 if i < extra else 0) for i in range(4)]
        self.PIECE_NT = nts              # [25, 25, 24, 24]
        self.PIECE_T0 = [sum(nts[:i]) for i in range(4)]
        self.PIECE_ROWS = [nt * 128 for nt in nts]
        self.SEGROWS = [ncores * r for r in self.PIECE_ROWS]
        assert all(s <= 32767 for s in self.SEGROWS)
        self.NSEG = 4


FULL = Cfg(N=100000, E=1600000, F=64, H=64, C=16, G=64)


# ---------------------------------------------------------------- host prep
def host_prep(cfg, x, edge_index, batch):
    N, G = cfg.N, cfg.G
    ncores, TILES, NPC = cfg.ncores, cfg.TILES, cfg.NPC
    NSEG = cfg.NSEG
    src = np.asarray(edge_index[0], dtype=np.int64)
    dst = np.asarray(edge_index[1], dtype=np.int64)
    batch = np.asarray(batch, dtype=np.int64)

    deg = np.bincount(src, minlength=N).astype(np.float64)
    dinv = np.where(deg > 0, 1.0 / np.sqrt(np.maximum(deg, 1.0)), 0.0).astype(np.float32)

    # ---- deal nodes into (core, tile) bins, balancing in-degree ----
    indeg = np.bincount(dst, minlength=N)
    order = np.argsort(-indeg, kind="stable")
    nbins = ncores * TILES
    k = np.arange(N)
    rnd = k // nbins
    pos = k % nbins
    binid = np.where(rnd % 2 == 0, pos, nbins - 1 - pos)
    core_of_bin = binid % ncores
    tile_of_bin = binid // ncores
    g_of_sorted = core_of_bin * NPC + tile_of_bin * 128 + rnd
    dealt = np.empty(N, dtype=np.int64)
    dealt[order] = g_of_sorted

    src_g = dealt[src]
    dst_g = dealt[dst]

    # per-node (dealt) attributes
    dinv_d = np.zeros(cfg.NPAD, dtype=np.float32)
    dinv_d[dealt] = dinv
    batch_d = np.full(cfg.NPAD, -1.0, dtype=np.float32)
    batch_d[dealt] = batch.astype(np.float32)
    x_d = np.zeros((cfg.NPAD, cfg.F), dtype=np.float32)
    x_d[dealt] = np.asarray(x, dtype=np.float32)

    cnt = np.bincount(batch, minlength=G).astype(np.float32)
    cinv = np.where(cnt > 0, 1.0 / np.maximum(cnt, 1.0), 0.0).astype(np.float32)
    cnt01 = (cnt > 0).astype(np.float32)
    cinv_d = np.zeros(cfg.NPAD, dtype=np.float32)
    bidx = batch_d.astype(np.int64)
    cinv_d[bidx >= 0] = cinv[bidx[bidx >= 0]]

    # ---- edge organization: (dst core, dst tile, src piece) ----
    e_core = dst_g // NPC
    e_tile = (dst_g % NPC) // 128
    e_drel = dst_g % 128
    s_tile = (src_g % NPC) // 128
    s_core = src_g // NPC
    s_slot = src_g % 128
    t0s = np.array(cfg.PIECE_T0)
    e_seg = np.searchsorted(t0s, s_tile, side="right") - 1
    rows_p = np.array(cfg.PIECE_ROWS)[e_seg]
    e_idx = s_core * rows_p + (s_tile - t0s[e_seg]) * 128 + s_slot

    order_e = np.lexsort((src_g, e_seg, e_tile, e_core))
    e_core, e_tile, e_drel, e_seg, e_idx = (a[order_e] for a in
                                            (e_core, e_tile, e_drel, e_seg, e_idx))
    dst_go = dst_g[order_e]

    gid = ((e_core * TILES + e_tile) * NSEG + e_seg).astype(np.int64)
    counts = np.bincount(gid, minlength=ncores * TILES * NSEG).reshape(ncores, TILES, NSEG)
    chunk_tbl = -(-counts.max(axis=0) // 128)  # [TILES, NSEG]

    blocks = [list(range(b, min(b + cfg.BLOCK, TILES))) for b in range(0, TILES, cfg.BLOCK)]
    regions = []       # (seg, slot_off, n_slots) -- one per (block, seg), one gather each
    ts_off = np.zeros((TILES, NSEG), dtype=np.int64)
    off = 0
    for blk in blocks:
        for s in range(NSEG):
            g_off = off
            for t in blk:
                ts_off[t, s] = off
                off += int(chunk_tbl[t, s]) * 128
            if off > g_off:
                regions.append((s, g_off, off - g_off))
    TOT = off
    assert TOT % 128 == 0

    # place each core's edges into slots
    idx_all = np.zeros((ncores, TOT), dtype=np.int16)
    drel_all = np.full((ncores, TOT), -1.0, dtype=np.float32)
    grp_start = np.zeros(ncores * TILES * NSEG, dtype=np.int64)
    np.cumsum(counts.reshape(-1)[:-1], out=grp_start[1:])
    within = np.arange(len(gid)) - grp_start[gid]
    slot_of_edge = ts_off[e_tile, e_seg] + within
    for c in range(ncores):
        m = e_core == c
        idx_all[c, slot_of_edge[m]] = e_idx[m].astype(np.int16)
        drel_all[c, slot_of_edge[m]] = e_drel[m].astype(np.float32)

    # wrapped layouts
    idx_wrapped = np.ascontiguousarray(
        np.tile(idx_all.reshape(ncores, TOT // 16, 16).transpose(0, 2, 1), (1, 8, 1))
    )  # [ncores, 128, TOT//16]
    drel_w = drel_all.reshape(ncores, TOT // 128, 128).transpose(0, 2, 1)

    # pool-weight matrix (layer-3 elimination), rows = dealt node ids
    Wp = np.zeros((cfg.NPAD, G), np.float32)
    np.add.at(Wp, (src_g, batch[dst]), -dinv[dst])
    Wp *= cinv[None, :]

    # per-core node-attribute wraps: [128, TILES]
    dinv_wt = dinv_d.reshape(ncores, TILES, 128).transpose(0, 2, 1)
    batch_wt = batch_d.reshape(ncores, TILES, 128).transpose(0, 2, 1)
    cinv_wt = cinv_d.reshape(ncores, TILES, 128).transpose(0, 2, 1)

    plan = dict(chunk_tbl=chunk_tbl, blocks=blocks, regions=regions,
                ts_off=ts_off, TOT=TOT)
    percore = dict(
        xT=[np.ascontiguousarray(x_d[c * NPC:(c + 1) * NPC].T) for c in range(ncores)],
        idx=[np.ascontiguousarray(idx_wrapped[c]) for c in range(ncores)],
        drel=[np.ascontiguousarray(drel_w[c]) for c in range(ncores)],
        Wp=[np.ascontiguousarray(Wp[c * NPC:(c + 1) * NPC]).astype(ml_dtypes.bfloat16)
            for c in range(ncores)],
        dinv=[np.ascontiguousarray(dinv_wt[c]) for c in range(ncores)],
        batch=[np.ascontiguousarray(batch_wt[c]) for c in range(ncores)],
        cinv=[np.ascontiguousarray(cinv_wt[c]) for c in range(ncores)],
    )
    return plan, percore, cnt01


# ---------------------------------------------------------------- program
def build_program(cfg, plan):
    TILES, NSEG, NPC = cfg.TILES, cfg.NSEG, cfg.NPC
    F, H, C, G = cfg.F, cfg.H, cfg.C, cfg.G
    chunk_tbl = plan["chunk_tbl"]; blocks = plan["blocks"]
    regions = plan["regions"]; ts_off = plan["ts_off"]; TOT = plan["TOT"]
    P_T0, P_NT = cfg.PIECE_T0, cfg.PIECE_NT
    PIECE_ROWS, SEGROWS = cfg.PIECE_ROWS, cfg.SEGROWS
    piece_of_tile = np.searchsorted(np.array(P_T0), np.arange(TILES), side="right") - 1
    piece_end_tile = [P_T0[p] + P_NT[p] - 1 for p in range(4)]

    # max chunks per (block,seg) region -> fixed msg tile shapes
    CMAXB = {s: 1 for s in range(NSEG)}
    for (s, goff, n) in regions:
        CMAXB[s] = max(CMAXB[s], n // 128)

    nc = bacc.Bacc(num_devices=cfg.ncores, target_bir_lowering=False, num_swdge_queues=4)

    # ---- I/O -----------------------------------------------------------
    P = {}
    P["xT"] = nc.declare_dram_parameter("xT", [F, NPC], BF16, isOutput=False)
    P["idx"] = nc.declare_dram_parameter("idx", [128, TOT // 16], I16, isOutput=False)
    P["drel"] = nc.declare_dram_parameter("drel", [128, TOT // 128], BF16, isOutput=False)
    P["Wp"] = nc.declare_dram_parameter("Wp", [NPC, G], BF16, isOutput=False)
    P["dinv"] = nc.declare_dram_parameter("dinv", [128, TILES], F32, isOutput=False)
    P["batch"] = nc.declare_dram_parameter("batch", [128, TILES], F32, isOutput=False)
    P["cinv"] = nc.declare_dram_parameter("cinv", [128, TILES], F32, isOutput=False)
    for l in range(3):
        P[f"Wa{l}"] = nc.declare_dram_parameter(f"Wa{l}", [F if l == 0 else H, H], BF16, isOutput=False)
        P[f"Wb{l}"] = nc.declare_dram_parameter(f"Wb{l}", [F if l == 0 else H, H], BF16, isOutput=False)
        P[f"bias{l}"] = nc.declare_dram_parameter(f"bias{l}", [1, H], BF16, isOutput=False)
    P["Wlin"] = nc.declare_dram_parameter("Wlin", [H, C], F32, isOutput=False)
    P["blin"] = nc.declare_dram_parameter("blin", [1, C], F32, isOutput=False)
    P["cnt01"] = nc.declare_dram_parameter("cnt01", [1, G], BF16, isOutput=False)
    P["iota"] = nc.declare_dram_parameter("iota", [128, 128], BF16, isOutput=False)
    P["identb"] = nc.declare_dram_parameter("identb", [128, 128], BF16, isOutput=False)
    out_ext = nc.declare_dram_parameter("out", [G, C], F32, isOutput=True)

    # internal DRAM: per-piece AG in/out (out double-buffered per layer)
    y_self = [nc.dram_tensor(f"y_self{p}", [PIECE_ROWS[p], 2 * H], BF16)
              for p in range(4)]
    y_piece = [[nc.dram_tensor(f"y_piece{li}_{p}", [SEGROWS[p], 2 * H], BF16,
                               addr_space="Shared") for p in range(4)]
               for li in range(2)]
    pool_in = nc.dram_tensor("pool_in", [H, G], F32)
    pool_out = nc.dram_tensor("pool_out", [H, G], F32, addr_space="Shared")

    with tile.TileContext(nc) as tc:
        with tc.tile_pool(name="const", bufs=1) as cpool, \
             tc.tile_pool(name="state", bufs=1) as spool, \
             tc.tile_pool(name="work", bufs=3) as wpool, \
             tc.tile_pool(name="msgs", bufs=2) as mpool, \
             tc.tile_pool(name="oh", bufs=2) as ohpool, \
             tc.tile_pool(name="psS", bufs=2, space="PSUM") as psS, \
             tc.tile_pool(name="psT", bufs=2, space="PSUM") as psT, \
             tc.tile_pool(name="psY", bufs=2, space="PSUM") as psY, \
             tc.tile_pool(name="psPZ", bufs=1, space="PSUM") as psPZ, \
             tc.tile_pool(name="psPH", bufs=1, space="PSUM") as psPH:

            # ---- load constants ----
            def cload(name, shape, dt=F32):
                t = cpool.tile(shape, dt, tag=name)
                nc.sync.dma_start(out=t[:], in_=P[name][:, :])
                return t

            iota_t = cload("iota", [128, 128], BF16)
            identb_t = cload("identb", [128, 128], BF16)
            dinv_t = cload("dinv", [128, TILES])
            ndinv_t = cpool.tile([128, TILES], F32, tag="ndinv")
            nc.vector.tensor_scalar(out=ndinv_t[:], in0=dinv_t[:], scalar1=-1.0,
                                    scalar2=None, op0=mybir.AluOpType.mult)
            batch_t = cload("batch", [128, TILES])
            cinv_t = cload("cinv", [128, TILES])
            drel_t = cload("drel", [128, TOT // 128], BF16)
            idx_t = cpool.tile([128, TOT // 16], I16, tag="idx")
            nc.sync.dma_start(out=idx_t[:], in_=P["idx"][:, :])
            cnt01_t = cload("cnt01", [1, G], BF16)
            Wa, Wb, bias = [], [], []
            for l in range(3):
                Wa.append(cload(f"Wa{l}", [F if l == 0 else H, H], BF16))
                Wb.append(cload(f"Wb{l}", [F if l == 0 else H, H], BF16))
                bias.append(cload(f"bias{l}", [1, H], BF16))
            wlin_t = cload("Wlin", [H, C])
            blin_t = cload("blin", [1, C])
            onesb_t = cpool.tile([1, 128], BF16, tag="onesb")
            nc.gpsimd.memset(onesb_t[:], 1.0)
            ones_t = cpool.tile([1, 128], F32, tag="ones")
            nc.gpsimd.memset(ones_t[:], 1.0)
            # zero the pad halves of y_self rows once
            zpad_t = cpool.tile([128, H], BF16, tag="zpad")
            nc.vector.memset(zpad_t[:], 0.0)
            for p in range(4):
                for tt in range(P_NT[p]):
                    nc.sync.dma_start(out=y_self[p][tt * 128:(tt + 1) * 128, H:2 * H],
                                      in_=zpad_t[:])

            # persistent transposed node state: layer-0 = x^T (one big load),
            # layer-1 = h1^T written per tile
            hT0_all = cpool.tile([F, TILES * 128], BF16, tag="hT0")
            nc.sync.dma_start(out=hT0_all[:], in_=P["xT"][:, :])
            hT1 = [spool.tile([F, 128], BF16, tag=f"hT1_{t}", name=f"hT1_{t}")
                   for t in range(TILES)]

            def hT_slice(a, t):
                return hT0_all[:, t * 128:(t + 1) * 128] if a == 0 else hT1[t][:]

            psum_pz = psPZ.tile([H, G], F32, tag="pz")
            psum_ph = psPH.tile([H, G], F32, tag="ph")

            def y_prep(l, t, hT_ap):
                """y = dinv*(h@Wb[l]) for tile t -> y_self piece; AG when piece done."""
                ps_y = psY.tile([128, H], F32, tag="y", name="ps_y")
                nc.tensor.matmul(ps_y[:], hT_ap, Wb[l][:], start=True, stop=True)
                y_sb = wpool.tile([128, H], BF16, tag="ysb", name="y_sb")
                nc.scalar.activation(y_sb[:], ps_y[:], mybir.ActivationFunctionType.Copy,
                                     scale=dinv_t[:, t:t + 1])
                p = int(piece_of_tile[t])
                tt = t - P_T0[p]
                nc.sync.dma_start(out=y_self[p][tt * 128:(tt + 1) * 128, 0:H], in_=y_sb[:])
                li = l  # y for SpMM layer l reads buffer set l
                if t == piece_end_tile[p]:
                    nc.gpsimd.collective_compute(
                        "AllGather", mybir.AluOpType.bypass,
                        replica_groups=[list(range(cfg.ncores))],
                        ins=[y_self[p][:, :].opt()], outs=[y_piece[li][p][:, :].opt()],
                    )

            # ---------- L0 prep: y1 pieces straight from x^T ----------
            for t in range(TILES):
                y_prep(0, t, hT0_all[:, t * 128:(t + 1) * 128])

            # ---------- SpMM layers (li = 0, 1) ----------
            for li in range(2):
                ri = 0
                qn = 0
                for blk in blocks:
                    blk_msgs = {}
                    for s in range(NSEG):
                        n_g = sum(int(chunk_tbl[t, s]) * 128 for t in blk)
                        if n_g == 0:
                            continue
                        (rs, roff, rn) = regions[ri]
                        assert rs == s and rn == n_g, (rs, s, rn, n_g, ri)
                        ri += 1
                        nck_r = n_g // 128
                        m_t = mpool.tile([128, CMAXB[s], 2 * H], BF16, tag=f"m{s}")
                        # split into <=1024-slot windows (descriptor carveout:
                        # dynamic_dma_scratch_size // 16 = 1024 descs per queue)
                        nwin = -(-nck_r // 8)
                        base = nck_r // nwin
                        rem = nck_r % nwin
                        w0 = 0
                        for wi in range(nwin):
                            wc = base + (1 if wi < rem else 0)
                            wn = wc * 128
                            woff = roff + w0 * 128
                            nc.gpsimd.dma_gather(
                                m_t[:, w0:w0 + wc, :],
                                y_piece[li][s][0:SEGROWS[s], :],
                                idx_t[:, woff // 16:(woff + wn) // 16],
                                wn, wn, 2 * H, queue_num=qn)
                            qn = (qn + 1) % 4
                            w0 += wc
                        # one-hot for the whole region in one DVE op:
                        # oh[p, c, j] = (iota[j] == drel[p, c0+c])
                        oh_r = ohpool.tile([128, CMAXB[s], 128], BF16, tag=f"oh{s}")
                        c0 = roff // 128
                        nc.vector.tensor_tensor(
                            out=oh_r[:, 0:nck_r, :],
                            in0=iota_t[:].unsqueeze(1).broadcast_to([128, nck_r, 128]),
                            in1=drel_t[:, c0:c0 + nck_r].unsqueeze(2)
                                .broadcast_to([128, nck_r, 128]),
                            op=mybir.AluOpType.is_equal)
                        blk_msgs[s] = (m_t, oh_r, roff)

                    for t in blk:
                        ps_d = psS.tile([128, H], F32, tag="s", name="ps_d")
                        nc.tensor.matmul(ps_d[:], hT_slice(li, t), Wa[li][:],
                                         start=True, stop=False)
                        nc.tensor.matmul(ps_d[:], onesb_t[:], bias[li][:],
                                         start=False, stop=True)
                        d_sb = wpool.tile([128, H], F32, tag="dsb")
                        nc.scalar.activation(d_sb[:], ps_d[:],
                                             mybir.ActivationFunctionType.Copy)
                        ps_s = psS.tile([128, H], F32, tag="s", name="ps_s")
                        nch = int(chunk_tbl[t].sum())
                        ci = 0
                        for s in range(NSEG):
                            nck = int(chunk_tbl[t, s])
                            if nck == 0:
                                continue
                            m_t, oh_r, roff2 = blk_msgs[s]
                            lo = (int(ts_off[t, s]) - roff2) // 128
                            for ck in range(nck):
                                nc.tensor.matmul(
                                    ps_s[:], oh_r[:, lo + ck, :], m_t[:, lo + ck, 0:H],
                                    start=(ci == 0), stop=(ci == nch - 1))
                                ci += 1
                        h_pre = wpool.tile([128, H], F32, tag="hpre")
                        nc.vector.scalar_tensor_tensor(
                            out=h_pre[:], in0=ps_s[:], scalar=ndinv_t[:, t:t + 1],
                            in1=d_sb[:], op0=mybir.AluOpType.mult,
                            op1=mybir.AluOpType.add)
                        h_sb = wpool.tile([128, H], BF16, tag="hsb")
                        nc.scalar.activation(h_sb[:], h_pre[:],
                                             mybir.ActivationFunctionType.Relu)
                        ps_t = psT.tile([H, 128], BF16, tag="tr", name="ps_t")
                        nc.tensor.transpose(ps_t[:], h_sb[:], identb_t[:])
                        if li == 0:
                            nc.vector.tensor_copy(hT1[t][:], ps_t[:])
                            y_prep(li + 1, t, hT1[t][:])
                        else:
                            # L2 tail: z = dinv*(h2@W3b); pool accumulations
                            hTc = wpool.tile([H, 128], BF16, tag="hTc")
                            nc.vector.tensor_copy(hTc[:], ps_t[:])
                            ps_y = psY.tile([128, H], F32, tag="y", name="ps_y")
                            nc.tensor.matmul(ps_y[:], hTc[:], Wb[2][:],
                                             start=True, stop=True)
                            z_sb = wpool.tile([128, H], BF16, tag="zsb")
                            nc.scalar.activation(z_sb[:], ps_y[:],
                                                 mybir.ActivationFunctionType.Copy,
                                                 scale=dinv_t[:, t:t + 1])
                            wp_t = wpool.tile([128, G], BF16, tag="wp")
                            nc.sync.dma_start(out=wp_t[:],
                                              in_=P["Wp"][t * 128:(t + 1) * 128, :])
                            poh_t = ohpool.tile([128, G], BF16, tag="poh")
                            nc.vector.tensor_scalar(
                                out=poh_t[:], in0=iota_t[:, :G],
                                scalar1=batch_t[:, t:t + 1],
                                scalar2=cinv_t[:, t:t + 1],
                                op0=mybir.AluOpType.is_equal,
                                op1=mybir.AluOpType.mult)
                            nc.tensor.matmul(psum_ph[:], h_sb[:], poh_t[:],
                                             start=(t == 0), stop=(t == TILES - 1),
                                             skip_group_check=True)
                            nc.tensor.matmul(psum_pz[:], z_sb[:], wp_t[:],
                                             start=(t == 0), stop=False,
                                             skip_group_check=True)
                assert ri == len(regions), (ri, len(regions))

            # ---------- finish pooled^T = psum_pz + W3a^T @ P2T + b3 x cnt01 ----------
            p2t_sb = wpool.tile([H, G], BF16, tag="p2t")
            nc.vector.tensor_copy(p2t_sb[:], psum_ph[:])
            nc.tensor.matmul(psum_pz[:], Wa[2][:], p2t_sb[:],
                             start=False, stop=False, skip_group_check=True)
            nc.tensor.matmul(psum_pz[:], bias[2][:], cnt01_t[:],
                             start=False, stop=True, skip_group_check=True)
            pl_sb = wpool.tile([H, G], F32, tag="plsb")
            nc.vector.tensor_copy(pl_sb[:], psum_pz[:])
            nc.sync.dma_start(out=pool_in[:, :], in_=pl_sb[:])
            nc.gpsimd.collective_compute(
                "AllReduce", mybir.AluOpType.add,
                replica_groups=[list(range(cfg.ncores))],
                ins=[pool_in[:, :].opt()], outs=[pool_out[:, :].opt()],
            )
            arT = wpool.tile([H, G], F32, tag="arT")
            nc.sync.dma_start(out=arT[:], in_=pool_out[:, :])
            ps_yo = psY.tile([128, H], F32, tag="y", name="ps_y")
            ps_o = ps_yo[0:G, 0:C]
            nc.tensor.matmul(ps_o, arT[:], wlin_t[:], start=True, stop=False)
            nc.tensor.matmul(ps_o, ones_t[:, :G], blin_t[:], start=False, stop=True)
            out_sb = wpool.tile([G, C], F32, tag="outsb")
            nc.vector.tensor_copy(out_sb[:], ps_o)
            nc.sync.dma_start(out=out_ext[:, :], in_=out_sb[:])

    nc.compile()
    return nc


# ---------------------------------------------------------------- driver
def make_in_maps(cfg, percore, cnt01, W1, b1, W2, b2, W3, b3, Wlin, blin):
    iota = np.tile(np.arange(128, dtype=np.float32)[None, :], (128, 1))
    ident = np.eye(128, dtype=np.float32)  # identb only
    Ws = [np.asarray(W1, np.float32), np.asarray(W2, np.float32), np.asarray(W3, np.float32)]
    bs = [np.asarray(b1, np.float32), np.asarray(b2, np.float32), np.asarray(b3, np.float32)]
    bf = ml_dtypes.bfloat16
    in_maps = []
    for c in range(cfg.ncores):
        m = {
            "xT": percore["xT"][c].astype(bf),
            "idx": percore["idx"][c],
            "drel": percore["drel"][c].astype(bf),
            "Wp": percore["Wp"][c],
            "dinv": percore["dinv"][c],
            "batch": percore["batch"][c],
            "cinv": percore["cinv"][c],
            "cnt01": cnt01[None, :].astype(bf),
            "iota": iota.astype(bf),
            "identb": ident.astype(bf),
            "Wlin": np.ascontiguousarray(Wlin, dtype=np.float32),
            "blin": np.ascontiguousarray(blin, dtype=np.float32)[None, :],
        }
        for l in range(3):
            m[f"Wa{l}"] = np.ascontiguousarray(Ws[l][0]).astype(bf)
            m[f"Wb{l}"] = np.ascontiguousarray(Ws[l][1]).astype(bf)
            m[f"bias{l}"] = np.ascontiguousarray(bs[l])[None, :].astype(bf)
        in_maps.append(m)
    return in_maps


def run(cfg, inputs, trace=False):
    plan, percore, cnt01 = host_prep(cfg, inputs["x"], inputs["edge_index"], inputs["batch"])
    nc = build_program(cfg, plan)
    in_maps = make_in_maps(cfg, percore, cnt01,
                           inputs["W1"], inputs["b1"], inputs["W2"], inputs["b2"],
                           inputs["W3"], inputs["b3"], inputs["Wlin"], inputs["blin"])
    res = run_bass_kernel_spmd(nc, in_maps, core_ids=list(range(cfg.ncores)), trace=trace)
    return np.asarray(res.results[0]["out"]), res


def kernel(**inputs) -> np.ndarray:
    out, _ = run(FULL, inputs, trace=False)
    return out
